# revision 65
# baseline (speedup 1.0000x reference)
"""Causal self-attention (B=4, T=2048, C=1024, H=16) on 8 TRN2 NeuronCores.

Sharding: 2 cores per batch element; each core computes 8 of the 16 heads
(tensor parallel) for its batch: QKV projection, causal attention, and a
partial output projection. The host sums the two partial outputs per batch.

v3 design (vs v2):
 - Cost-model-driven pacing: the emitter tracks an estimated PE clock P and
   a FIFO chain A[k] of predicted exp completion times (one entry per S
   pack, matching the 2-slot ssp PSUM ring). Before emitting S pack k it
   weaves filler groups until P >= A[k-2], so the PE never out-runs the
   Activation engine's softmax-exp throughput (the late-schedule pacer).
 - Fine-grained fillers: QKV accumulation groups are split into 4-chunk
   halves (~850ns) and the output projection into nt-halves (~850ns), so
   pacing gaps between S packs can be plugged exactly.
 - Filler rebalance: proj(0) weaves into phase 2; proj(1..3) into phase 3
   (the most exp-deficient); 2 packs/head of tiles 2 and 3 are prepacked
   one phase early (same SBUF allocation as v2's pp tags).
 - Chunk-major DMA-paced startup: wq/x0 stream in 2-chunk transfers and the
   first QKV accumulates chunk-major across 4 open PSUM groups, so the PE
   starts ~4.3us in and never waits for a full weight tensor.
 - Tail: the final projection quarter emits split 512-col y DMAs.
"""

import collections

import numpy as np
import ml_dtypes

import concourse.bacc as bacc
import concourse.mybir as mybir
import concourse.tile as tile
import concourse.bass_utils as bass_utils
from concourse.bass_interp import get_hw_module

B, T, C = 4, 2048, 1024
H = 16          # total heads
D = C // H      # 64
HPC = 8         # heads per core
N_CORES = 8

FP = mybir.dt.float32
BF = mybir.dt.bfloat16
BF_NP = ml_dtypes.bfloat16

# pacing cost model (ns)
PE_NS = 1.0 / 2.4      # per output column, bf16
ACT_NS = 1.0 / 1.2     # per activation column
ACT_OVH = 240.0        # per-activation-op fixed overhead
SEM = 150.0            # semaphore propagation margin
SSP_DEPTH = 2          # ssp PSUM ring depth

_CACHE = {}


def build_nc():
    nc = bacc.Bacc("TRN2", target_bir_lowering=False, debug=False,
                   num_devices=N_CORES)

    xt = nc.dram_tensor("xt", [C, T], BF, kind="ExternalInput").ap()
    wq = nc.dram_tensor("wq", [C, 512], BF, kind="ExternalInput").ap()
    wk = nc.dram_tensor("wk", [C, 512], BF, kind="ExternalInput").ap()
    wv = nc.dram_tensor("wv", [C, 512], BF, kind="ExternalInput").ap()
    wp = nc.dram_tensor("wp", [512, C], BF, kind="ExternalInput").ap()
    mask = nc.dram_tensor("mask", [128, 128], BF, kind="ExternalInput").ap()
    ident = nc.dram_tensor("ident", [128, 128], BF, kind="ExternalInput").ap()
    y = nc.dram_tensor("y", [T, C], BF, kind="ExternalOutput").ap()

    EXP = mybir.ActivationFunctionType.Exp
    SCALE = 1.0 / np.sqrt(D)
    mm = nc.tensor.matmul

    with tile.TileContext(nc) as tc:
        with tc.tile_pool(name="big", bufs=1) as big, \
             tc.tile_pool(name="xsp", bufs=2) as xsp, \
             tc.tile_pool(name="ptp", bufs=10) as ptp, \
             tc.tile_pool(name="otp", bufs=8) as otp, \
             tc.tile_pool(name="sml", bufs=2) as sml, \
             tc.tile_pool(name="ysp", bufs=2) as ysp, \
             tc.tile_pool(name="mmp", bufs=2, space="PSUM") as mmp, \
             tc.tile_pool(name="ssp", bufs=2, space="PSUM") as ssp, \
             tc.tile_pool(name="pvp", bufs=1, space="PSUM") as pvp, \
             tc.tile_pool(name="trp", bufs=1, space="PSUM") as trp:

            # ---------------- persistent tiles ----------------
            mask_t = big.tile([128, 128], BF, name="mask_t")
            ident_t = big.tile([128, 128], BF, name="ident_t")
            ones_t = big.tile([128, 8], BF, name="ones_t")
            nc.vector.memset(ones_t[:], 1.0)

            # KT is persistent (every later q-tile attends all earlier keys)
            KT = [big.tile([128, T], BF, name=f"kt{p}") for p in range(4)]
            # (tile, pair) -> [128, 512] ring tiles; each lives ~2 phases
            # (written in phase j-1 via fillers, consumed in phase j/j+1)
            QTS, OTS = {}, {}
            VG = [big.tile([128, HPC * (D + 1)], BF, name=f"vg{i}")
                  for i in range(T // 128)]
            WQ, WK, WV, WP = [None] * 8, [None] * 8, [None] * 8, [None] * 8

            # ---------------- prologue DMAs --------------------
            # wq and x(tile 0) stream in interleaved 2-chunk transfers so the
            # chunk-major first QKV can start ~4.3us in; wk follows in
            # 2-chunk transfers; wv/wp batched (needed later).
            wqw = big.tile([128, 4096], BF, name="wideq")
            xw0 = xsp.tile([128, 4096], BF, name="xw0", tag="xs")
            wkw = big.tile([128, 4096], BF, name="widek")
            # mixed-size transfers: single chunk first (low latency), then
            # growing batches so HWDGE setup amortizes while arrival still
            # tracks the chunk-major consumption rate
            for lo, hi in ((0, 1), (1, 3), (3, 5), (5, 8)):
                n = hi - lo
                csl = slice(lo * 512, hi * 512)
                rsl = slice(lo * 128, hi * 128)
                nc.sync.dma_start(
                    wqw[:, csl].rearrange("p (c n) -> p c n", c=n),
                    wq[rsl, :].rearrange("(c p) n -> p c n", p=128))
                nc.sync.dma_start(
                    xw0[:, csl].rearrange("p (c t) -> p c t", c=n),
                    xt[rsl, 0:512].rearrange("(c p) t -> p c t", p=128))
            for hv in range(4):
                csl = slice(hv * 2 * 512, (hv + 1) * 2 * 512)
                rsl = slice(hv * 2 * 128, (hv + 1) * 2 * 128)
                nc.sync.dma_start(
                    wkw[:, csl].rearrange("p (c n) -> p c n", c=2),
                    wk[rsl, :].rearrange("(c p) n -> p c n", p=128))
            nc.sync.dma_start(mask_t[:], mask[:])
            nc.sync.dma_start(ident_t[:], ident[:])

            def batch_w(dst_tiles, wsrc, nm):
                wide = big.tile([128, 4096], BF, name=f"wide{nm}")
                nc.sync.dma_start(
                    wide[:].rearrange("p (c n) -> p c n", c=8),
                    wsrc[:].rearrange("(c p) n -> p c n", p=128))
                for cc in range(8):
                    dst_tiles[cc] = wide[:, cc * 512:(cc + 1) * 512]

            for cc in range(8):
                WQ[cc] = wqw[:, cc * 512:(cc + 1) * 512]
                WK[cc] = wkw[:, cc * 512:(cc + 1) * 512]
            batch_w(WV, wv[:], "v")
            wpw = big.tile([128, 4096], BF, name="widewp")
            nc.sync.dma_start(
                wpw[:].rearrange("p (c n) -> p c n", c=4),
                wp[:].rearrange("(c p) n -> p c n", p=128))
            for i in range(8):
                c2, nt = i // 2, i % 2
                WP[i] = wpw[:, c2 * 1024 + nt * 512:c2 * 1024 + nt * 512 + 512]
            xs0 = [xw0[:, cc * 512:(cc + 1) * 512] for cc in range(8)]

            # ---------------- pacing state ----------------
            state = {"P": 0.0}
            exp_done = []                   # A[k] per emitted S pack
            fillers = collections.deque()   # (pe_est, pack_key, closure)
            pending = collections.deque()   # (ready_P, pe_est, closure)
            qk_done = set()                 # (tile, pair) with QT+KT written
            FEED_LOW = 2200.0

            def bump(ns):
                state["P"] += ns

            def run_filler(nonpack=False):
                if pending and state["P"] >= pending[0][0]:
                    _, pe, cl = pending.popleft()
                    cl()
                    bump(pe)
                    return True
                if fillers:
                    if nonpack and fillers[0][1] is not None:
                        # skip pack fillers while trying to raise P for a
                        # pack slot (would recurse on the same constraint)
                        for i, (pe, key, cl) in enumerate(fillers):
                            if key is None:
                                del fillers[i]
                                cl()
                                bump(pe)
                                return True
                        return False
                    pe, key, cl = fillers.popleft()
                    cl()
                    bump(pe)
                    return True
                return False

            def weave_until(target):
                while state["P"] < target:
                    if not run_filler(nonpack=True):
                        break

            def feed_act():
                """Keep ACT fed: when the exp chain is about to run dry,
                emit the first dependency-ready prepack from the queue."""
                a_tail = exp_done[-1] if exp_done else 0.0
                if a_tail >= state["P"] + FEED_LOW:
                    return
                for i, (pe, key, cl) in enumerate(fillers):
                    if key is not None and key in qk_done:
                        del fillers[i]
                        cl()
                        bump(pe)
                        return

            def drain(force_pending=False):
                while fillers:
                    if not run_filler():
                        break
                while force_pending and pending:
                    _, pe, cl = pending.popleft()
                    cl()
                    bump(pe)

            # ---------------- compute groups ----------------
            def load_x_batched(rt):
                wide = xsp.tile([128, 4096], BF, name=f"xw{rt}", tag="xs")
                nc.sync.dma_start(
                    wide[:].rearrange("p (c t) -> p c t", c=8),
                    xt[:, rt * 512:(rt + 1) * 512]
                    .rearrange("(c p) t -> p c t", p=128))
                return [wide[:, cc * 512:(cc + 1) * 512] for cc in range(8)]

            def qk_half(p, w_t, rt, xs, half, ps_box, is_k):
                """Half of a QKV accumulation group: chunks 4*half..4*half+3.
                ps_box carries the PSUM tile across the two halves."""
                def run():
                    psl = slice(p * 128, (p + 1) * 128)
                    if half == 0:
                        ps_box[0] = mmp.tile([128, 512], FP,
                                             name=f"q{rt}{p}", tag="mm")
                    ps = ps_box[0]
                    for cc in range(4 * half, 4 * half + 4):
                        mm(ps[:], w_t[cc][:, psl], xs[cc][:],
                           start=(cc == 0), stop=(cc == 7))
                    if half == 1:
                        if is_k:
                            nc.vector.tensor_copy(
                                KT[p][:, rt * 512:(rt + 1) * 512], ps[:])
                            qk_done.add((rt, p))
                        else:
                            QTS[rt, p] = otp.tile([128, 512], BF,
                                                  name=f"qt{rt}{p}",
                                                  tag="qt")
                            nc.vector.tensor_copy(QTS[rt, p][:], ps[:])
                return run

            def v_half(rc, rt, xs, half, ps_box):
                def run():
                    if half == 0:
                        ps_box[0] = mmp.tile([128, 512], FP,
                                             name=f"v{rt}{rc}", tag="mm")
                    ps = ps_box[0]
                    for cc in range(4 * half, 4 * half + 4):
                        mm(ps[:],
                           xs[cc][:, rc * 128:(rc + 1) * 128],
                           WV[cc][:],
                           start=(cc == 0), stop=(cc == 7))
                    if half == 1:
                        i = rt * 4 + rc
                        vgv = VG[i][:].rearrange("p (h e) -> p h e", h=HPC)
                        nc.vector.tensor_copy(
                            vgv[:, :, 0:D],
                            ps[:].rearrange("p (h d) -> p h d", h=HPC))
                        nc.vector.tensor_copy(
                            vgv[:, :, D:D + 1],
                            ones_t[:].rearrange("p (h o) -> p h o", h=8))
                return run

            def qkv_fillers(rt, xs):
                """24 ~850ns filler closures computing QKV for tile rt."""
                groups = []
                for p in range(4):
                    bq, bk = [None], [None]
                    groups.append((853.0, None,
                                   qk_half(p, WQ, rt, xs, 0, bq, False)))
                    groups.append((853.0, None,
                                   qk_half(p, WQ, rt, xs, 1, bq, False)))
                    groups.append((853.0, None,
                                   qk_half(p, WK, rt, xs, 0, bk, True)))
                    groups.append((853.0, None,
                                   qk_half(p, WK, rt, xs, 1, bk, True)))
                for rc in range(4):
                    bv = [None]
                    groups.append((853.0, None, v_half(rc, rt, xs, 0, bv)))
                    groups.append((853.0, None, v_half(rc, rt, xs, 1, bv)))
                return groups

            def proj_nt(j, qc, nt, yt_box, split_dma=False, fine=False):
                """nt-half of the output projection of token block qc."""
                def run():
                    qcs = slice(qc * 128, (qc + 1) * 128)
                    if nt == 0:
                        yt_box[0] = ysp.tile([128, 1024], BF,
                                             name=f"y{qc}", tag="yst")
                    yt = yt_box[0]
                    pr = mmp.tile([128, 512], FP, name=f"pr{qc}{nt}",
                                  tag="mm")
                    lqs = slice((qc - 4 * j) * 128, (qc - 4 * j + 1) * 128)
                    for c2 in range(4):
                        mm(pr[:], OTS[j, c2][:, lqs], WP[c2 * 2 + nt][:],
                           start=(c2 == 0), stop=(c2 == 3))
                    o = nt * 512
                    if fine:
                        # quarter-granularity copies+DMAs shrink the final
                        # copy->transfer->sem chain at the very end
                        for q4 in range(2):
                            ys = slice(o + q4 * 256, o + (q4 + 1) * 256)
                            nc.vector.tensor_copy(
                                yt[:, ys], pr[:, q4 * 256:(q4 + 1) * 256])
                            nc.sync.dma_start(y[qcs, ys], yt[:, ys])
                        return
                    nc.vector.tensor_copy(yt[:, o:o + 512], pr[:])
                    if split_dma:
                        nc.sync.dma_start(y[qcs, o:o + 512],
                                          yt[:, o:o + 512])
                    elif nt == 1:
                        nc.sync.dma_start(y[qcs, :], yt[:])
                return run

            def proj_fillers(j):
                groups = []
                for qc in range(4 * j, 4 * j + 4):
                    box = [None]
                    split = (j == 3 and qc >= 14)
                    groups.append((853.0, None,
                                   proj_nt(j, qc, 0, box, split_dma=split)))
                    groups.append((853.0, None,
                                   proj_nt(j, qc, 1, box, split_dma=split)))
                return groups

            # ---------------- attention ----------------
            pre_pt = {}   # (j, h, kc) -> (pt, off, q0, a_est)

            def emit_pack(j, h, pi, pack, tag="pt"):
                """Emit S matmuls + exp for one pack, paced against ACT."""
                k = len(exp_done)
                if k >= SSP_DEPTH:
                    weave_until(exp_done[k - SSP_DEPTH])
                p, hh = h // 2, h % 2
                dsl = slice(hh * 64, hh * 64 + 64)
                ext = pack[-1][1] + pack[-1][3]
                s_ps = ssp.tile([128, 1024], FP, name=f"s{j}{h}{pi}",
                                tag="s")
                for (kc, off, q0, nv) in pack:
                    mm(s_ps[:, off:off + nv],
                       KT[p][dsl, kc * 128:(kc + 1) * 128],
                       QTS[j, p][dsl, q0:512],
                       start=True, stop=True)
                bump(ext * PE_NS)
                a_prev = exp_done[-1] if exp_done else 0.0
                a = max(state["P"] + SEM, a_prev) + ext * ACT_NS + ACT_OVH
                exp_done.append(a)
                pt = ptp.tile([128, 1024], BF, name=f"p{j}{h}{pi}",
                              tag=tag,
                              bufs={"pt": 8, "pp2": 10, "pp3": 32}[tag])
                nc.scalar.activation(pt[:, 0:ext], s_ps[:, 0:ext], EXP,
                                     scale=SCALE)
                for (kc, off, q0, nv) in pack:
                    if kc >= 4 * j:   # diagonal: mask first 128 cols
                        nc.vector.tensor_mul(pt[:, off:off + 128],
                                             pt[:, off:off + 128],
                                             mask_t[:])
                    pre_pt[j, h, kc] = (pt, off, q0, a)

            def make_packs(j):
                packs = [[(2 * t, 0, 0, 512), (2 * t + 1, 512, 0, 512)]
                         for t in range(2 * j)]
                packs.append([(4 * j, 0, 0, 512),
                              (4 * j + 1, 512, 128, 384)])
                packs.append([(4 * j + 2, 0, 256, 256),
                              (4 * j + 3, 256, 384, 128)])
                return packs

            def prepack_fillers(j, counts):
                """Pack-fillers computing the first counts[h] S+exp packs of
                each head of q-tile j one phase early. Grouped per head-pair
                so they can slot right after that pair's QKV closures."""
                per_pair = [[] for _ in range(4)]
                packs = make_packs(j)
                for h in range(HPC):
                    for pi in range(counts[h]):
                        def run(j=j, h=h, pi=pi, pack=packs[pi]):
                            emit_pack(j, h, pi, pack, tag=f"pp{j}")
                        ext = packs[pi][-1][1] + packs[pi][-1][3]
                        per_pair[h // 2].append(
                            (ext * PE_NS, (j, h // 2), run))
                return per_pair

            stg_tiles = {}

            def head_packs(j, h, hold_last):
                """Emit this head's live S packs (all but the last when
                hold_last, so the in-flight pt window stays within the
                ring while the previous head's PV is still pending)."""
                packs = make_packs(j)
                live = [(pi, pk) for pi, pk in enumerate(packs)
                        if (j, h, pk[0][0]) not in pre_pt]
                tail = live[-1:] if hold_last else []
                for pi, pk in (live[:-1] if hold_last else live):
                    emit_pack(j, h, pi, pk)
                    feed_act()
                return tail

            def head_finish(j, h, tail):
                for pi, pk in tail:
                    emit_pack(j, h, pi, pk)
                    feed_act()
                return {kc: pre_pt.pop((j, h, kc))
                        for kc in range(4 * j + 4)}

            def head_pv(j, h, ptmap):
                p, hh = h // 2, h % 2
                # qc-major PV; gate each qc on the exp of its newest pack.
                last_head = (j == 3 and h == HPC - 1)
                if hh == 1:
                    OTS[j, p] = otp.tile([128, 512], BF, name=f"ots{j}{p}",
                                         tag="ot")
                pv = pvp.tile([128, 512], FP, name=f"pv{j}{h}", tag="pv")

                def emit_pv_qc(qc):
                    weave_until(max(ptmap[kc][3]
                                    for kc in range(4 * j + qc + 1)))
                    for kc in range(4 * j + qc + 1):
                        pt, off, q0, _ = ptmap[kc]
                        cl = qc * 128 - q0
                        mm(pv[:, qc * 65:qc * 65 + 65],
                           pt[:, off + cl:off + cl + 128],
                           VG[kc][:, h * 65:(h + 1) * 65],
                           start=(kc == 0), stop=(kc == 4 * j + qc))
                    bump((4 * j + qc + 1) * 65 * PE_NS)
                    feed_act()

                if last_head:
                    # tail: pipeline PV(qc+1) with normalize+transpose of
                    # qc, then immediately project+store that token block.
                    stg_t = stg_tiles[p]
                    ov = stg_t[:].rearrange("p (q c) -> p q c", c=128)

                    def finish_qc(qc):
                        oc1 = sml.tile([128, 65], FP, name=f"ocq{qc}",
                                       tag="ocq", bufs=4)
                        nc.vector.tensor_copy(oc1[:],
                                              pv[:, qc * 65:qc * 65 + 65])
                        rc1 = sml.tile([128, 1], FP, name=f"rcq{qc}",
                                       tag="rcq", bufs=4)
                        nc.vector.reciprocal(rc1[:], oc1[:, 64:65])
                        nc.vector.tensor_scalar_mul(
                            ov[:, qc, 64:128], oc1[:, 0:64], rc1[:, 0:1])
                        tr = trp.tile([128, 128], BF, name=f"tl{qc}",
                                      tag="tr")
                        nc.tensor.transpose(
                            tr[:], stg_t[:, qc * 128:(qc + 1) * 128],
                            ident_t[:])
                        nc.vector.tensor_copy(
                            OTS[j, p][:, qc * 128:(qc + 1) * 128], tr[:])
                        bump(55.0)
                        for pe_est, _, clo in proj3[2 * qc:2 * qc + 2]:
                            clo()
                            bump(pe_est)

                    prev_qc = None
                    for qc in range(4):
                        emit_pv_qc(qc)
                        if prev_qc is not None:
                            finish_qc(prev_qc)
                        prev_qc = qc
                    finish_qc(3)
                    return

                for qc in range(4):
                    emit_pv_qc(qc)

                # evacuate PV to SBUF once (frees the single-buffer PSUM
                # bank for the next head), then normalize from the copy.
                ocp = sml.tile([128, 260], FP, name=f"oc{j}{h}", tag="ocp",
                               bufs=2)
                nc.vector.tensor_copy(ocp[:], pv[:, 0:260])
                pvv = ocp[:].rearrange("p (q e) -> p q e", e=65)
                rec = sml.tile([128, 4], FP, name=f"rc{j}{h}", tag="rec",
                               bufs=2)
                recv = rec[:].rearrange("p (q e) -> p q e", e=1)
                nc.vector.reciprocal(recv, pvv[:, :, 64:65])
                if hh == 0:
                    stg_tiles[p] = sml.tile([128, 512], BF, name=f"sg{j}{p}",
                                            tag="stg", bufs=2)
                stg_t = stg_tiles[p]
                ov = stg_t[:].rearrange("p (q c) -> p q c", c=128)
                for qc in range(4):
                    nc.vector.tensor_scalar_mul(
                        ov[:, qc, hh * 64:hh * 64 + 64],
                        pvv[:, qc, 0:64], rec[:, qc:qc + 1])
                if hh == 1:
                    ready = state["P"] + 2000.0   # DVE normalize chain
                    ot_t = OTS[j, p]
                    for qc in range(4):
                        def tr_op(p=p, qc=qc, stg_t=stg_t, ot_t=ot_t, j=j):
                            tr = trp.tile([128, 128], BF, name=f"t{j}{p}{qc}",
                                          tag="tr")
                            nc.tensor.transpose(
                                tr[:], stg_t[:, qc * 128:(qc + 1) * 128],
                                ident_t[:])
                            nc.vector.tensor_copy(
                                ot_t[:, qc * 128:(qc + 1) * 128], tr[:])
                        pending.append((ready, 55.0, tr_op))

            # ---------------- startup: chunk-major QKV(0) ----------------
            # 4 open PSUM groups per tensor (mmp's two banks + the two
            # 512-col banks of one ssp tile), accumulated chunk-pair-major
            # so compute tracks the 2-chunk DMA arrivals.
            def qkv0_chunk_major(w_t, is_k, tag):
                sp = ssp.tile([128, 1024], FP, name=f"cm{tag}", tag="s")
                boxes = [mmp.tile([128, 512], FP, name=f"cm{tag}{p}",
                                  tag="mm") for p in range(2)]
                boxes += [sp[:, 0:512], sp[:, 512:1024]]
                for cc in range(8):
                    for p in range(4):
                        psl = slice(p * 128, (p + 1) * 128)
                        mm(boxes[p][:], w_t[cc][:, psl], xs0[cc][:],
                           start=(cc == 0), stop=(cc == 7))
                bump(4 * 4096 * PE_NS)
                for p in range(4):
                    if is_k:
                        nc.vector.tensor_copy(KT[p][:, 0:512], boxes[p][:])
                    else:
                        QTS[0, p] = otp.tile([128, 512], BF,
                                             name=f"qt0{p}", tag="qt")
                        nc.vector.tensor_copy(QTS[0, p][:], boxes[p][:])

            qkv0_chunk_major(WQ, False, "q")
            qkv0_chunk_major(WK, True, "k")
            for rc in range(4):
                bv = [None]
                v_half(rc, 0, xs0, 0, bv)()
                v_half(rc, 0, xs0, 1, bv)()
                bump(2 * 853.0)

            # ---------------- main schedule ----------------
            proj3 = proj_fillers(3)
            for j in range(4):
                if j < 3:
                    xs = load_x_batched(j + 1)
                    qkv = qkv_fillers(j + 1, xs)
                    # prepacks of tile j+1 slot right after the qk fillers
                    # of their head-pair (which produce their QT/KT inputs)
                    if j == 1:
                        pre = prepack_fillers(2, [2, 2, 1, 1, 1, 1, 1, 1])
                    elif j == 2:
                        pre = prepack_fillers(3, [4] * 8)
                    else:
                        pre = [[] for _ in range(4)]
                    for p in range(4):
                        fillers.extend(qkv[4 * p:4 * p + 4])
                        fillers.extend(pre[p])
                    fillers.extend(qkv[16:])
                if j == 1:
                    fillers.extend(proj_fillers(0))
                if j == 2:
                    fillers.extend(proj_fillers(1))
                if j == 3:
                    fillers.extend(proj_fillers(2))
                prev = None
                for h in range(HPC):
                    tail = head_packs(j, h, hold_last=(prev is not None))
                    if prev is not None:
                        head_pv(j, prev[0], prev[1])
                    ptmap = head_finish(j, h, tail)
                    prev = (h, ptmap)
                if j == 3:
                    # flush pending transposes before the last head's PV so
                    # the final projection never waits on stale pendings
                    drain(force_pending=True)
                head_pv(j, prev[0], prev[1])
                drain(force_pending=True)

    nc.compile()
    nc.m = get_hw_module(nc.m)
    return nc


def _make_mask():
    k = np.arange(128)[:, None]
    t = np.arange(128)[None, :]
    return (t >= k).astype(BF_NP)


def _make_ident():
    return np.eye(128, dtype=BF_NP)


def make_in_maps(x, w_attn, w_proj):
    mask = _make_mask()
    ident = _make_ident()
    in_maps = []
    for c in range(N_CORES):
        b, g = c // 2, c % 2
        gs = slice(g * 512, (g + 1) * 512)
        in_maps.append({
            "xt": np.ascontiguousarray(x[b].T).astype(BF_NP),
            "wq": np.ascontiguousarray(w_attn[:, 0 * C:1 * C][:, gs]).astype(BF_NP),
            "wk": np.ascontiguousarray(w_attn[:, 1 * C:2 * C][:, gs]).astype(BF_NP),
            "wv": np.ascontiguousarray(w_attn[:, 2 * C:3 * C][:, gs]).astype(BF_NP),
            "wp": np.ascontiguousarray(w_proj[gs, :]).astype(BF_NP),
            "mask": mask,
            "ident": ident,
        })
    return in_maps


def kernel(x, w_attn, w_proj):
    x = np.ascontiguousarray(x, dtype=np.float32)
    w_attn = np.ascontiguousarray(w_attn, dtype=np.float32)
    w_proj = np.ascontiguousarray(w_proj, dtype=np.float32)

    if "nc" not in _CACHE:
        _CACHE["nc"] = build_nc()
    nc = _CACHE["nc"]

    in_maps = make_in_maps(x, w_attn, w_proj)
    res = bass_utils.run_bass_kernel_spmd(
        nc, in_maps, core_ids=list(range(N_CORES)))

    y = np.empty((B, T, C), dtype=np.float32)
    for b in range(B):
        y[b] = (res.results[2 * b]["y"].astype(np.float32)
                + res.results[2 * b + 1]["y"].astype(np.float32))
    return y


# revision 69
# speedup vs baseline: 1.0057x; 1.0057x over previous
"""Causal self-attention (B=4, T=2048, C=1024, H=16) on 8 TRN2 NeuronCores.

Sharding: 2 cores per batch element; each core computes 8 of the 16 heads
(tensor parallel) for its batch: QKV projection, causal attention, and a
partial output projection. The host sums the two partial outputs per batch.

v3 design (vs v2):
 - Cost-model-driven pacing: the emitter tracks an estimated PE clock P and
   a FIFO chain A[k] of predicted exp completion times (one entry per S
   pack, matching the 2-slot ssp PSUM ring). Before emitting S pack k it
   weaves filler groups until P >= A[k-2], so the PE never out-runs the
   Activation engine's softmax-exp throughput (the late-schedule pacer).
 - Fine-grained fillers: QKV accumulation groups are split into 4-chunk
   halves (~850ns) and the output projection into nt-halves (~850ns), so
   pacing gaps between S packs can be plugged exactly.
 - Filler rebalance: proj(0) weaves into phase 2; proj(1..3) into phase 3
   (the most exp-deficient); 2 packs/head of tiles 2 and 3 are prepacked
   one phase early (same SBUF allocation as v2's pp tags).
 - Chunk-major DMA-paced startup: wq/x0 stream in 2-chunk transfers and the
   first QKV accumulates chunk-major across 4 open PSUM groups, so the PE
   starts ~4.3us in and never waits for a full weight tensor.
 - Tail: the final projection quarter emits split 512-col y DMAs.
"""

import collections

import numpy as np
import ml_dtypes

import concourse.bacc as bacc
import concourse.mybir as mybir
import concourse.tile as tile
import concourse.bass_utils as bass_utils
from concourse.bass_interp import get_hw_module

B, T, C = 4, 2048, 1024
H = 16          # total heads
D = C // H      # 64
HPC = 8         # heads per core
N_CORES = 8

FP = mybir.dt.float32
BF = mybir.dt.bfloat16
BF_NP = ml_dtypes.bfloat16

# pacing cost model (ns)
PE_NS = 1.0 / 2.4      # per output column, bf16
ACT_NS = 1.0 / 1.2     # per activation column
ACT_OVH = 240.0        # per-activation-op fixed overhead
SEM = 150.0            # semaphore propagation margin
SSP_DEPTH = 2          # ssp PSUM ring depth

_CACHE = {}


def build_nc():
    nc = bacc.Bacc("TRN2", target_bir_lowering=False, debug=False,
                   num_devices=N_CORES)

    xt = nc.dram_tensor("xt", [C, T], BF, kind="ExternalInput").ap()
    wq = nc.dram_tensor("wq", [C, 512], BF, kind="ExternalInput").ap()
    wk = nc.dram_tensor("wk", [C, 512], BF, kind="ExternalInput").ap()
    wv = nc.dram_tensor("wv", [C, 512], BF, kind="ExternalInput").ap()
    wp = nc.dram_tensor("wp", [512, C], BF, kind="ExternalInput").ap()
    mask = nc.dram_tensor("mask", [128, 128], BF, kind="ExternalInput").ap()
    ident = nc.dram_tensor("ident", [128, 128], BF, kind="ExternalInput").ap()
    y = nc.dram_tensor("y", [T, C], BF, kind="ExternalOutput").ap()

    EXP = mybir.ActivationFunctionType.Exp
    SCALE = 1.0 / np.sqrt(D)
    mm = nc.tensor.matmul

    with tile.TileContext(nc) as tc:
        with tc.tile_pool(name="big", bufs=1) as big, \
             tc.tile_pool(name="xsp", bufs=2) as xsp, \
             tc.tile_pool(name="ptp", bufs=10) as ptp, \
             tc.tile_pool(name="otp", bufs=8) as otp, \
             tc.tile_pool(name="sml", bufs=2) as sml, \
             tc.tile_pool(name="ysp", bufs=2) as ysp, \
             tc.tile_pool(name="mmp", bufs=2, space="PSUM") as mmp, \
             tc.tile_pool(name="ssp", bufs=2, space="PSUM") as ssp, \
             tc.tile_pool(name="pvp", bufs=1, space="PSUM") as pvp, \
             tc.tile_pool(name="trp", bufs=1, space="PSUM") as trp:

            # ---------------- persistent tiles ----------------
            mask_t = big.tile([128, 128], BF, name="mask_t")
            ident_t = big.tile([128, 128], BF, name="ident_t")
            ones_t = big.tile([128, 8], BF, name="ones_t")
            nc.vector.memset(ones_t[:], 1.0)

            # KT is persistent (every later q-tile attends all earlier keys)
            KT = [big.tile([128, T], BF, name=f"kt{p}") for p in range(4)]
            # (tile, pair) -> [128, 512] ring tiles; each lives ~2 phases
            # (written in phase j-1 via fillers, consumed in phase j/j+1)
            QTS, OTS = {}, {}
            VG = [big.tile([128, HPC * (D + 1)], BF, name=f"vg{i}")
                  for i in range(T // 128)]
            WQ, WK, WV, WP = [None] * 8, [None] * 8, [None] * 8, [None] * 8

            # ---------------- prologue DMAs --------------------
            # wq and x(tile 0) stream in interleaved 2-chunk transfers so the
            # chunk-major first QKV can start ~4.3us in; wk follows in
            # 2-chunk transfers; wv/wp batched (needed later).
            wqw = big.tile([128, 4096], BF, name="wideq")
            xw0 = xsp.tile([128, 4096], BF, name="xw0", tag="xs")
            wkw = big.tile([128, 4096], BF, name="widek")
            # mixed-size transfers: single chunk first (low latency), then
            # growing batches so HWDGE setup amortizes while arrival still
            # tracks the chunk-major consumption rate
            for lo, hi in ((0, 1), (1, 3), (3, 5), (5, 8)):
                n = hi - lo
                csl = slice(lo * 512, hi * 512)
                rsl = slice(lo * 128, hi * 128)
                nc.sync.dma_start(
                    wqw[:, csl].rearrange("p (c n) -> p c n", c=n),
                    wq[rsl, :].rearrange("(c p) n -> p c n", p=128))
                nc.sync.dma_start(
                    xw0[:, csl].rearrange("p (c t) -> p c t", c=n),
                    xt[rsl, 0:512].rearrange("(c p) t -> p c t", p=128))
            for hv in range(4):
                csl = slice(hv * 2 * 512, (hv + 1) * 2 * 512)
                rsl = slice(hv * 2 * 128, (hv + 1) * 2 * 128)
                nc.sync.dma_start(
                    wkw[:, csl].rearrange("p (c n) -> p c n", c=2),
                    wk[rsl, :].rearrange("(c p) n -> p c n", p=128))
            nc.sync.dma_start(mask_t[:], mask[:])
            nc.sync.dma_start(ident_t[:], ident[:])

            def batch_w(dst_tiles, wsrc, nm):
                wide = big.tile([128, 4096], BF, name=f"wide{nm}")
                nc.sync.dma_start(
                    wide[:].rearrange("p (c n) -> p c n", c=8),
                    wsrc[:].rearrange("(c p) n -> p c n", p=128))
                for cc in range(8):
                    dst_tiles[cc] = wide[:, cc * 512:(cc + 1) * 512]

            for cc in range(8):
                WQ[cc] = wqw[:, cc * 512:(cc + 1) * 512]
                WK[cc] = wkw[:, cc * 512:(cc + 1) * 512]
            batch_w(WV, wv[:], "v")
            wpw = big.tile([128, 4096], BF, name="widewp")
            nc.sync.dma_start(
                wpw[:].rearrange("p (c n) -> p c n", c=4),
                wp[:].rearrange("(c p) n -> p c n", p=128))
            for i in range(8):
                c2, nt = i // 2, i % 2
                WP[i] = wpw[:, c2 * 1024 + nt * 512:c2 * 1024 + nt * 512 + 512]
            xs0 = [xw0[:, cc * 512:(cc + 1) * 512] for cc in range(8)]

            # ---------------- pacing state ----------------
            state = {"P": 0.0}
            exp_done = []                   # A[k] per emitted S pack
            fillers = collections.deque()   # (pe_est, pack_key, closure)
            pending = collections.deque()   # (ready_P, pe_est, closure)
            qk_done = set()                 # (tile, pair) with QT+KT written
            FEED_LOW = 1500.0

            def bump(ns):
                state["P"] += ns

            def run_filler(nonpack=False):
                if pending and state["P"] >= pending[0][0]:
                    _, pe, cl = pending.popleft()
                    cl()
                    bump(pe)
                    return True
                if fillers:
                    if nonpack and fillers[0][1] is not None:
                        # skip pack fillers while trying to raise P for a
                        # pack slot (would recurse on the same constraint)
                        for i, (pe, key, cl) in enumerate(fillers):
                            if key is None:
                                del fillers[i]
                                cl()
                                bump(pe)
                                return True
                        return False
                    pe, key, cl = fillers.popleft()
                    cl()
                    bump(pe)
                    return True
                return False

            def weave_until(target):
                while state["P"] < target:
                    if not run_filler(nonpack=True):
                        break

            def feed_act():
                """Keep ACT fed: when the exp chain is about to run dry,
                emit the first dependency-ready prepack from the queue."""
                a_tail = exp_done[-1] if exp_done else 0.0
                if a_tail >= state["P"] + FEED_LOW:
                    return
                for i, (pe, key, cl) in enumerate(fillers):
                    if key is not None and key in qk_done:
                        del fillers[i]
                        cl()
                        bump(pe)
                        return

            def drain(force_pending=False):
                while fillers:
                    if not run_filler():
                        break
                while force_pending and pending:
                    _, pe, cl = pending.popleft()
                    cl()
                    bump(pe)

            # ---------------- compute groups ----------------
            def load_x_batched(rt):
                wide = xsp.tile([128, 4096], BF, name=f"xw{rt}", tag="xs")
                nc.sync.dma_start(
                    wide[:].rearrange("p (c t) -> p c t", c=8),
                    xt[:, rt * 512:(rt + 1) * 512]
                    .rearrange("(c p) t -> p c t", p=128))
                return [wide[:, cc * 512:(cc + 1) * 512] for cc in range(8)]

            def qk_half(p, w_t, rt, xs, half, ps_box, is_k):
                """Half of a QKV accumulation group: chunks 4*half..4*half+3.
                ps_box carries the PSUM tile across the two halves."""
                def run():
                    psl = slice(p * 128, (p + 1) * 128)
                    if half == 0:
                        ps_box[0] = mmp.tile([128, 512], FP,
                                             name=f"q{rt}{p}", tag="mm")
                    ps = ps_box[0]
                    for cc in range(4 * half, 4 * half + 4):
                        mm(ps[:], w_t[cc][:, psl], xs[cc][:],
                           start=(cc == 0), stop=(cc == 7))
                    if half == 1:
                        if is_k:
                            nc.vector.tensor_copy(
                                KT[p][:, rt * 512:(rt + 1) * 512], ps[:])
                            qk_done.add((rt, p))
                        else:
                            QTS[rt, p] = otp.tile([128, 512], BF,
                                                  name=f"qt{rt}{p}",
                                                  tag="qt")
                            nc.vector.tensor_copy(QTS[rt, p][:], ps[:])
                return run

            def v_half(rc, rt, xs, half, ps_box):
                def run():
                    if half == 0:
                        ps_box[0] = mmp.tile([128, 512], FP,
                                             name=f"v{rt}{rc}", tag="mm")
                    ps = ps_box[0]
                    for cc in range(4 * half, 4 * half + 4):
                        mm(ps[:],
                           xs[cc][:, rc * 128:(rc + 1) * 128],
                           WV[cc][:],
                           start=(cc == 0), stop=(cc == 7))
                    if half == 1:
                        i = rt * 4 + rc
                        vgv = VG[i][:].rearrange("p (h e) -> p h e", h=HPC)
                        nc.vector.tensor_copy(
                            vgv[:, :, 0:D],
                            ps[:].rearrange("p (h d) -> p h d", h=HPC))
                        nc.vector.tensor_copy(
                            vgv[:, :, D:D + 1],
                            ones_t[:].rearrange("p (h o) -> p h o", h=8))
                return run

            def qkv_fillers(rt, xs):
                """24 ~850ns filler closures computing QKV for tile rt."""
                groups = []
                for p in range(4):
                    bq, bk = [None], [None]
                    groups.append((853.0, None,
                                   qk_half(p, WQ, rt, xs, 0, bq, False)))
                    groups.append((853.0, None,
                                   qk_half(p, WQ, rt, xs, 1, bq, False)))
                    groups.append((853.0, None,
                                   qk_half(p, WK, rt, xs, 0, bk, True)))
                    groups.append((853.0, None,
                                   qk_half(p, WK, rt, xs, 1, bk, True)))
                for rc in range(4):
                    bv = [None]
                    groups.append((853.0, None, v_half(rc, rt, xs, 0, bv)))
                    groups.append((853.0, None, v_half(rc, rt, xs, 1, bv)))
                return groups

            def proj_nt(j, qc, nt, yt_box, split_dma=False, fine=False):
                """nt-half of the output projection of token block qc."""
                def run():
                    qcs = slice(qc * 128, (qc + 1) * 128)
                    if nt == 0:
                        yt_box[0] = ysp.tile([128, 1024], BF,
                                             name=f"y{qc}", tag="yst")
                    yt = yt_box[0]
                    pr = mmp.tile([128, 512], FP, name=f"pr{qc}{nt}",
                                  tag="mm")
                    lqs = slice((qc - 4 * j) * 128, (qc - 4 * j + 1) * 128)
                    for c2 in range(4):
                        mm(pr[:], OTS[j, c2][:, lqs], WP[c2 * 2 + nt][:],
                           start=(c2 == 0), stop=(c2 == 3))
                    o = nt * 512
                    if fine:
                        # quarter-granularity copies+DMAs shrink the final
                        # copy->transfer->sem chain at the very end
                        for q4 in range(2):
                            ys = slice(o + q4 * 256, o + (q4 + 1) * 256)
                            nc.vector.tensor_copy(
                                yt[:, ys], pr[:, q4 * 256:(q4 + 1) * 256])
                            nc.sync.dma_start(y[qcs, ys], yt[:, ys])
                        return
                    nc.vector.tensor_copy(yt[:, o:o + 512], pr[:])
                    if split_dma:
                        nc.sync.dma_start(y[qcs, o:o + 512],
                                          yt[:, o:o + 512])
                    elif nt == 1:
                        nc.sync.dma_start(y[qcs, :], yt[:])
                return run

            def proj_fillers(j):
                groups = []
                for qc in range(4 * j, 4 * j + 4):
                    box = [None]
                    split = (j == 3 and qc >= 14)
                    groups.append((853.0, None,
                                   proj_nt(j, qc, 0, box, split_dma=split)))
                    groups.append((853.0, None,
                                   proj_nt(j, qc, 1, box, split_dma=split)))
                return groups

            # ---------------- attention ----------------
            pre_pt = {}   # (j, h, kc) -> (pt, off, q0, a_est)

            def emit_pack(j, h, pi, pack, tag="pt"):
                """Emit S matmuls + exp for one pack, paced against ACT."""
                k = len(exp_done)
                if k >= SSP_DEPTH:
                    weave_until(exp_done[k - SSP_DEPTH])
                p, hh = h // 2, h % 2
                dsl = slice(hh * 64, hh * 64 + 64)
                ext = pack[-1][1] + pack[-1][3]
                s_ps = ssp.tile([128, 1024], FP, name=f"s{j}{h}{pi}",
                                tag="s")
                for (kc, off, q0, nv) in pack:
                    mm(s_ps[:, off:off + nv],
                       KT[p][dsl, kc * 128:(kc + 1) * 128],
                       QTS[j, p][dsl, q0:512],
                       start=True, stop=True)
                bump(ext * PE_NS)
                a_prev = exp_done[-1] if exp_done else 0.0
                a = max(state["P"] + SEM, a_prev) + ext * ACT_NS + ACT_OVH
                exp_done.append(a)
                pt = ptp.tile([128, 1024], BF, name=f"p{j}{h}{pi}",
                              tag=tag,
                              bufs={"pt": 8, "pp2": 12, "pp3": 30}[tag])
                nc.scalar.activation(pt[:, 0:ext], s_ps[:, 0:ext], EXP,
                                     scale=SCALE)
                for (kc, off, q0, nv) in pack:
                    if kc >= 4 * j:   # diagonal: mask first 128 cols
                        nc.vector.tensor_mul(pt[:, off:off + 128],
                                             pt[:, off:off + 128],
                                             mask_t[:])
                    pre_pt[j, h, kc] = (pt, off, q0, a)

            def make_packs(j):
                packs = [[(2 * t, 0, 0, 512), (2 * t + 1, 512, 0, 512)]
                         for t in range(2 * j)]
                packs.append([(4 * j, 0, 0, 512),
                              (4 * j + 1, 512, 128, 384)])
                packs.append([(4 * j + 2, 0, 256, 256),
                              (4 * j + 3, 256, 384, 128)])
                return packs

            def prepack_fillers(j, counts):
                """Pack-fillers computing the first counts[h] S+exp packs of
                each head of q-tile j one phase early. Grouped per head-pair
                so they can slot right after that pair's QKV closures."""
                per_pair = [[] for _ in range(4)]
                packs = make_packs(j)
                for h in range(HPC):
                    for pi in range(counts[h]):
                        def run(j=j, h=h, pi=pi, pack=packs[pi]):
                            emit_pack(j, h, pi, pack, tag=f"pp{j}")
                        ext = packs[pi][-1][1] + packs[pi][-1][3]
                        per_pair[h // 2].append(
                            (ext * PE_NS, (j, h // 2), run))
                return per_pair

            stg_tiles = {}

            def head_packs(j, h, hold_last):
                """Emit this head's live S packs (all but the last when
                hold_last, so the in-flight pt window stays within the
                ring while the previous head's PV is still pending)."""
                packs = make_packs(j)
                live = [(pi, pk) for pi, pk in enumerate(packs)
                        if (j, h, pk[0][0]) not in pre_pt]
                tail = live[-1:] if hold_last else []
                for pi, pk in (live[:-1] if hold_last else live):
                    emit_pack(j, h, pi, pk)
                    feed_act()
                return tail

            def head_finish(j, h, tail):
                for pi, pk in tail:
                    emit_pack(j, h, pi, pk)
                    feed_act()
                return {kc: pre_pt.pop((j, h, kc))
                        for kc in range(4 * j + 4)}

            def head_pv(j, h, ptmap):
                p, hh = h // 2, h % 2
                # qc-major PV; gate each qc on the exp of its newest pack.
                last_head = (j == 3 and h == HPC - 1)
                if hh == 1:
                    OTS[j, p] = otp.tile([128, 512], BF, name=f"ots{j}{p}",
                                         tag="ot")
                pv = pvp.tile([128, 512], FP, name=f"pv{j}{h}", tag="pv")

                def emit_pv_qc(qc):
                    weave_until(max(ptmap[kc][3]
                                    for kc in range(4 * j + qc + 1)))
                    for kc in range(4 * j + qc + 1):
                        pt, off, q0, _ = ptmap[kc]
                        cl = qc * 128 - q0
                        mm(pv[:, qc * 65:qc * 65 + 65],
                           pt[:, off + cl:off + cl + 128],
                           VG[kc][:, h * 65:(h + 1) * 65],
                           start=(kc == 0), stop=(kc == 4 * j + qc))
                    bump((4 * j + qc + 1) * 65 * PE_NS)
                    feed_act()

                if last_head:
                    # tail: pipeline PV(qc+1) with normalize+transpose of
                    # qc, then immediately project+store that token block.
                    stg_t = stg_tiles[p]
                    ov = stg_t[:].rearrange("p (q c) -> p q c", c=128)

                    def finish_qc(qc):
                        oc1 = sml.tile([128, 65], FP, name=f"ocq{qc}",
                                       tag="ocq", bufs=4)
                        nc.vector.tensor_copy(oc1[:],
                                              pv[:, qc * 65:qc * 65 + 65])
                        rc1 = sml.tile([128, 1], FP, name=f"rcq{qc}",
                                       tag="rcq", bufs=4)
                        nc.vector.reciprocal(rc1[:], oc1[:, 64:65])
                        nc.vector.tensor_scalar_mul(
                            ov[:, qc, 64:128], oc1[:, 0:64], rc1[:, 0:1])
                        tr = trp.tile([128, 128], BF, name=f"tl{qc}",
                                      tag="tr")
                        nc.tensor.transpose(
                            tr[:], stg_t[:, qc * 128:(qc + 1) * 128],
                            ident_t[:])
                        nc.vector.tensor_copy(
                            OTS[j, p][:, qc * 128:(qc + 1) * 128], tr[:])
                        bump(55.0)
                        for pe_est, _, clo in proj3[2 * qc:2 * qc + 2]:
                            clo()
                            bump(pe_est)

                    prev_qc = None
                    for qc in range(4):
                        emit_pv_qc(qc)
                        if prev_qc is not None:
                            finish_qc(prev_qc)
                        prev_qc = qc
                    finish_qc(3)
                    return

                for qc in range(4):
                    emit_pv_qc(qc)

                # evacuate PV to SBUF once (frees the single-buffer PSUM
                # bank for the next head), then normalize from the copy.
                ocp = sml.tile([128, 260], FP, name=f"oc{j}{h}", tag="ocp",
                               bufs=2)
                nc.vector.tensor_copy(ocp[:], pv[:, 0:260])
                pvv = ocp[:].rearrange("p (q e) -> p q e", e=65)
                rec = sml.tile([128, 4], FP, name=f"rc{j}{h}", tag="rec",
                               bufs=2)
                recv = rec[:].rearrange("p (q e) -> p q e", e=1)
                nc.vector.reciprocal(recv, pvv[:, :, 64:65])
                if hh == 0:
                    stg_tiles[p] = sml.tile([128, 512], BF, name=f"sg{j}{p}",
                                            tag="stg", bufs=2)
                stg_t = stg_tiles[p]
                ov = stg_t[:].rearrange("p (q c) -> p q c", c=128)
                for qc in range(4):
                    nc.vector.tensor_scalar_mul(
                        ov[:, qc, hh * 64:hh * 64 + 64],
                        pvv[:, qc, 0:64], rec[:, qc:qc + 1])
                if hh == 1:
                    ready = state["P"] + 2000.0   # DVE normalize chain
                    ot_t = OTS[j, p]
                    for qc in range(4):
                        def tr_op(p=p, qc=qc, stg_t=stg_t, ot_t=ot_t, j=j):
                            tr = trp.tile([128, 128], BF, name=f"t{j}{p}{qc}",
                                          tag="tr")
                            nc.tensor.transpose(
                                tr[:], stg_t[:, qc * 128:(qc + 1) * 128],
                                ident_t[:])
                            nc.vector.tensor_copy(
                                ot_t[:, qc * 128:(qc + 1) * 128], tr[:])
                        pending.append((ready, 55.0, tr_op))

            # ---------------- startup: chunk-major QKV(0) ----------------
            # 4 open PSUM groups per tensor (mmp's two banks + the two
            # 512-col banks of one ssp tile), accumulated chunk-pair-major
            # so compute tracks the 2-chunk DMA arrivals.
            def qkv0_chunk_major(w_t, is_k, tag):
                sp = ssp.tile([128, 1024], FP, name=f"cm{tag}", tag="s")
                boxes = [mmp.tile([128, 512], FP, name=f"cm{tag}{p}",
                                  tag="mm") for p in range(2)]
                boxes += [sp[:, 0:512], sp[:, 512:1024]]
                for cc in range(8):
                    for p in range(4):
                        psl = slice(p * 128, (p + 1) * 128)
                        mm(boxes[p][:], w_t[cc][:, psl], xs0[cc][:],
                           start=(cc == 0), stop=(cc == 7))
                bump(4 * 4096 * PE_NS)
                for p in range(4):
                    if is_k:
                        nc.vector.tensor_copy(KT[p][:, 0:512], boxes[p][:])
                    else:
                        QTS[0, p] = otp.tile([128, 512], BF,
                                             name=f"qt0{p}", tag="qt")
                        nc.vector.tensor_copy(QTS[0, p][:], boxes[p][:])

            qkv0_chunk_major(WQ, False, "q")
            qkv0_chunk_major(WK, True, "k")
            for rc in range(4):
                bv = [None]
                v_half(rc, 0, xs0, 0, bv)()
                v_half(rc, 0, xs0, 1, bv)()
                bump(2 * 853.0)

            # ---------------- main schedule ----------------
            proj3 = proj_fillers(3)
            for j in range(4):
                if j < 3:
                    xs = load_x_batched(j + 1)
                    qkv = qkv_fillers(j + 1, xs)
                    # prepacks of tile j+1 slot right after the qk fillers
                    # of their head-pair (which produce their QT/KT inputs)
                    if j == 1:
                        pre = prepack_fillers(2, [2, 2, 2, 2, 1, 1, 1, 1])
                    elif j == 2:
                        pre = prepack_fillers(3, [4, 4, 4, 4, 4, 4, 3, 3])
                    else:
                        pre = [[] for _ in range(4)]
                    for p in range(4):
                        fillers.extend(qkv[4 * p:4 * p + 4])
                        fillers.extend(pre[p])
                    fillers.extend(qkv[16:])
                if j == 1:
                    fillers.extend(proj_fillers(0))
                if j == 2:
                    fillers.extend(proj_fillers(1))
                if j == 3:
                    fillers.extend(proj_fillers(2))
                prev = None
                for h in range(HPC):
                    tail = head_packs(j, h, hold_last=(prev is not None))
                    if prev is not None:
                        head_pv(j, prev[0], prev[1])
                    ptmap = head_finish(j, h, tail)
                    prev = (h, ptmap)
                if j == 3:
                    # flush pending transposes before the last head's PV so
                    # the final projection never waits on stale pendings
                    drain(force_pending=True)
                head_pv(j, prev[0], prev[1])
                drain()
            drain(force_pending=True)

    nc.compile()
    nc.m = get_hw_module(nc.m)
    return nc


def _make_mask():
    k = np.arange(128)[:, None]
    t = np.arange(128)[None, :]
    return (t >= k).astype(BF_NP)


def _make_ident():
    return np.eye(128, dtype=BF_NP)


def make_in_maps(x, w_attn, w_proj):
    mask = _make_mask()
    ident = _make_ident()
    in_maps = []
    for c in range(N_CORES):
        b, g = c // 2, c % 2
        gs = slice(g * 512, (g + 1) * 512)
        in_maps.append({
            "xt": np.ascontiguousarray(x[b].T).astype(BF_NP),
            "wq": np.ascontiguousarray(w_attn[:, 0 * C:1 * C][:, gs]).astype(BF_NP),
            "wk": np.ascontiguousarray(w_attn[:, 1 * C:2 * C][:, gs]).astype(BF_NP),
            "wv": np.ascontiguousarray(w_attn[:, 2 * C:3 * C][:, gs]).astype(BF_NP),
            "wp": np.ascontiguousarray(w_proj[gs, :]).astype(BF_NP),
            "mask": mask,
            "ident": ident,
        })
    return in_maps


def kernel(x, w_attn, w_proj):
    x = np.ascontiguousarray(x, dtype=np.float32)
    w_attn = np.ascontiguousarray(w_attn, dtype=np.float32)
    w_proj = np.ascontiguousarray(w_proj, dtype=np.float32)

    if "nc" not in _CACHE:
        _CACHE["nc"] = build_nc()
    nc = _CACHE["nc"]

    in_maps = make_in_maps(x, w_attn, w_proj)
    res = bass_utils.run_bass_kernel_spmd(
        nc, in_maps, core_ids=list(range(N_CORES)))

    y = np.empty((B, T, C), dtype=np.float32)
    for b in range(B):
        y[b] = (res.results[2 * b]["y"].astype(np.float32)
                + res.results[2 * b + 1]["y"].astype(np.float32))
    return y
